# revision 13
# baseline (speedup 1.0000x reference)
"""SoftGate Trainium2 kernel.

nn_SoftGate: gate-MLP (Linear->ReLU->Linear->squashed tanh) over [B=8, S=4096,
D=1024] fp32, threshold mask, per-row stable compaction of kept tokens.

Strategy: data-parallel over batch across 8 NeuronCores (one row per core).
On each core:
  - stream x row token-major into SBUF (resident, 16 MB),
  - PE-transpose 128x128 blocks to get D-on-partitions for the matmul,
  - h = relu(x @ W1 (+ b1)) via 8 accumulating matmuls per 128-token tile in
    float32r (tf32, 1 cycle/row; native fp32 is 4 cycles/row on TRN2),
  - fused DVE scalar_tensor_tensor: (h max 0) * W2 with accum_out -> y per token,
  - tf32 boundary fixup: tokens with |y - C| < DELTA (C = threshold in pre-tanh
    space) get y recomputed exactly in native fp32 (indirect-DMA gather of those
    token rows, <= 2 per partition), so the keep/drop decision is fp32-exact,
  - gate y' = (1 + tanh(10 y + 10 b2)) / 2 on ACT; kept mask from pre-tanh y,
  - prefix sums via triangular matmuls on PE -> stable permutation dst (kept
    compact to front, rejected fill the tail backwards with zero values),
  - u = x * gate (DVE per-token scalar), indirect-DMA scatter of 4 KB token
    rows to v[dst].
The 'adjust' lift (only active when no token passes the threshold) is
speculated to be zero; the kernel reports each row's max gate and the fixup
overflow count so the host can fall back to an exact numpy recompute for rows
where speculation fails (never fires for this input distribution).
"""

import os
import sys

import numpy as np

for _p in ("/opt/trn_rl_repo",):
    if _p not in sys.path and os.path.isdir(_p):
        sys.path.insert(0, _p)

import concourse.bass as bass
import concourse.tile as tile
from bass_rust import add_dep_helper
from concourse import bacc, mybir
from concourse.bass_utils import run_bass_kernel_spmd

THRESHOLD = 0.1
EPS = 1e-05
B, S, D = 8, 4096, 1024
DH = 512
P = 128
NT = S // P          # 32 token tiles per row
NK = D // P          # 8 contraction chunks
F32 = mybir.dt.float32
F32R = mybir.dt.float32r
F16 = mybir.dt.float16
I32 = mybir.dt.int32

USE_TF32 = True
FIX_SLOTS = 1        # boundary tokens handled per partition
DELTA = 5e-3         # half-width of the recompute band around the threshold
BIG = 1.0e6          # out-of-bounds marker for empty fixup slots

# kept <=> (1 + tanh(10*y)) / 2 > THRESHOLD <=> y > atanh(2*THRESHOLD - 1) / 10
RAW_THRESHOLD = float(np.arctanh(2.0 * THRESHOLD - 1.0) / 10.0)


def _round_tf32(a):
    """Round-to-nearest-even to 10-bit mantissa (tf32), keeping fp32 storage."""
    v = a.astype(np.float32).view(np.uint32).astype(np.uint64)
    v = (v + 0x1000 + ((v >> 13) & 1)) & ~np.uint64(0x1FFF)
    return (v & 0xFFFFFFFF).astype(np.uint32).view(np.float32)


def _build_consts():
    """Small host-precomputed constant tensors (replicated to every core)."""
    ut128 = np.triu(np.ones((P, P), dtype=np.float32))          # ut[k, m] = 1 if k <= m
    ident = np.eye(P, dtype=np.float32)
    ones128 = np.ones((P, 1), dtype=np.float32)
    ones1x128 = np.ones((1, P), dtype=np.float32)
    # sut33[k, n] = 1 if k < n (n < 32); column 32 = all ones (grand total)
    sut33 = np.zeros((NT, NT + 1), dtype=np.float32)
    sut33[:, :NT] = np.triu(np.ones((NT, NT), dtype=np.float32), k=1)
    sut33[:, NT] = 1.0
    # token id t = c*128 + p laid out [p, c]; iotar = (S-1) - t
    t_idx = (np.arange(NT)[None, :] * P + np.arange(P)[:, None]).astype(np.float32)
    iotar = (S - 1) - t_idx
    onespn = np.ones((P, NT), dtype=np.float32)
    return {
        "c_ut128": ut128,
        "c_ident": ident,
        "c_ident16": ident.astype(np.float16),
        "c_ones128": ones128,
        "c_ones1x128": ones1x128,
        "c_sut33": sut33,
        "c_iotar": iotar,
        "c_tid": t_idx,
        "c_tidb": t_idx - BIG,
        "c_onespn": onespn,
    }


def build_module(b1_nonzero: bool):
    """Trace the single-core bass module. Returns the traced nc."""
    nc = bacc.Bacc(
        "TRN2",
        target_bir_lowering=False,
        debug=False,
        num_devices=8,
    )

    x_d = nc.dram_tensor("x_row", [S, D], F32, kind="ExternalInput")
    x16_d = None
    if USE_TF32:
        x16_d = nc.dram_tensor("x16_row", [S, D], F16, kind="ExternalInput")
    valid_d = nc.dram_tensor("valid_col", [P, NT], F32, kind="ExternalInput")
    # w1: host-rounded to fp16 (the gate matmul runs in fp16: same 11-bit
    # mantissa as tf32, but 2-byte weights halve the LDWEIGHTS cost)
    w1_d = nc.dram_tensor("w1", [D, DH], F16 if USE_TF32 else F32,
                          kind="ExternalInput")
    w1x_d = None
    if USE_TF32:
        w1x_d = nc.dram_tensor("w1x", [D, DH], F32, kind="ExternalInput")
    w2_d = nc.dram_tensor("w2r", [1, DH], F32, kind="ExternalInput")
    b2_d = nc.dram_tensor("b2r", [1, 1], F32, kind="ExternalInput")
    b1_d = None
    if b1_nonzero:
        b1_d = nc.dram_tensor("b1r", [1, DH], F32, kind="ExternalInput")

    c_ut128 = nc.dram_tensor("c_ut128", [P, P], F32, kind="ExternalInput")
    c_ident = nc.dram_tensor("c_ident", [P, P], F32, kind="ExternalInput")
    c_ident16 = nc.dram_tensor("c_ident16", [P, P], F16, kind="ExternalInput")
    c_ones128 = nc.dram_tensor("c_ones128", [P, 1], F32, kind="ExternalInput")
    c_ones1x128 = nc.dram_tensor("c_ones1x128", [1, P], F32, kind="ExternalInput")
    c_sut33 = nc.dram_tensor("c_sut33", [NT, NT + 1], F32, kind="ExternalInput")
    c_iotar = nc.dram_tensor("c_iotar", [P, NT], F32, kind="ExternalInput")
    c_tid = nc.dram_tensor("c_tid", [P, NT], F32, kind="ExternalInput")
    c_tidb = nc.dram_tensor("c_tidb", [P, NT], F32, kind="ExternalInput")
    c_onespn = nc.dram_tensor("c_onespn", [P, NT], F32, kind="ExternalInput")

    v_d = nc.dram_tensor("v_out", [S, D], F32, kind="ExternalOutput")
    stats_d = nc.dram_tensor("stats", [1, 4], F32, kind="ExternalOutput")
    yraw_d = nc.dram_tensor("yraw_out", [P, NT], F32, kind="ExternalOutput")

    with tile.TileContext(nc) as tc:
        with (
            tc.tile_pool(name="xres", bufs=NT) as xp,            # resident x row
            tc.tile_pool(name="consts", bufs=1) as cp,
            tc.tile_pool(name="xt", bufs=3) as xtp,              # transposed x tiles
            tc.tile_pool(name="xh", bufs=2) as xhp,              # fp16 x (4-tile batches)
            tc.tile_pool(name="hw", bufs=2) as hwp,              # fused-op dump
            tc.tile_pool(name="small", bufs=1) as sp,
            tc.tile_pool(name="fix", bufs=FIX_SLOTS) as fxp,     # gathered band rows
            tc.tile_pool(name="pst", bufs=4, space="PSUM") as pst,    # transpose banks
            tc.tile_pool(name="psh", bufs=2, space="PSUM") as psh,    # h accumulate
        ):
            # ---- constants / weights ----
            # Small constants first: the transposes + PE warm-up need ident/
            # ut128 immediately; the 2 MB W1 loads would otherwise block this
            # HWDGE FIFO for ~15us.
            ut128 = cp.tile([P, P], F32)
            nc.sync.dma_start(ut128[:], c_ut128[:, :])
            ident = cp.tile([P, P], F32)
            nc.sync.dma_start(ident[:], c_ident[:, :])
            ident16 = cp.tile([P, P], F16)
            nc.sync.dma_start(ident16[:], c_ident16[:, :])

            # PE warm-up: ~5us of dense matmul trips the HAM clock gate to 8/8
            warm_ps = pst.tile([P, 512], F32, tag="pst")
            for w in range(12):
                nc.tensor.matmul(
                    out=warm_ps[:, :P], lhsT=ident[:], rhs=ut128[:],
                    start=(w == 0), stop=(w == 11),
                )

            # W1 right behind the warm-up consts: the first h-matmul needs it
            w1_s = cp.tile([P, NK, DH], F16 if USE_TF32 else F32)
            nc.sync.dma_start(
                w1_s[:], w1_d.rearrange("(k p) h -> p k h", p=P)
            )
            xh_tiles = []
            if USE_TF32:
                # fp16 x in 1 MB batches of 4 token-tiles, straight after W1
                for g in range(NT // 4):
                    xh = xhp.tile([P, 4, D], F16, tag="xh")
                    xh_tiles.append(xh)
                    nc.sync.dma_start(
                        xh[:],
                        x16_d[g * 4 * P : (g + 1) * 4 * P, :].rearrange(
                            "(c p) d -> p c d", p=P
                        ),
                    )

            ones128 = cp.tile([P, 1], F32)
            nc.sync.dma_start(ones128[:], c_ones128[:, :])
            ones1x128 = cp.tile([1, P], F32)
            nc.sync.dma_start(ones1x128[:], c_ones1x128[:, :])
            sut33 = cp.tile([NT, NT + 1], F32)
            nc.sync.dma_start(sut33[:], c_sut33[:, :])
            iotar = cp.tile([P, NT], F32)
            nc.sync.dma_start(iotar[:], c_iotar[:, :])
            tid = cp.tile([P, NT], F32)
            nc.sync.dma_start(tid[:], c_tid[:, :])
            tidb = cp.tile([P, NT], F32)
            nc.sync.dma_start(tidb[:], c_tidb[:, :])
            onespn = cp.tile([P, NT], F32)
            nc.sync.dma_start(onespn[:], c_onespn[:, :])
            validf = cp.tile([P, NT], F32)
            nc.sync.dma_start(validf[:], valid_d[:, :])
            w2_s = cp.tile([P, DH], F32)              # W2 replicated on partitions
            nc.sync.dma_start(w2_s[:], w2_d[:, :].to_broadcast([P, DH]))
            b2_s = cp.tile([P, 1], F32)
            nc.sync.dma_start(b2_s[:], b2_d[:, :].to_broadcast([P, 1]))
            b2x10 = cp.tile([P, 1], F32)
            nc.scalar.mul(b2x10[:], b2_s[:], 10.0)
            if b1_nonzero:
                b1_s = cp.tile([1, DH], F32)
                nc.sync.dma_start(b1_s[:], b1_d[:, :])

            if USE_TF32:
                w1x_s = cp.tile([P, NK, DH], F32)     # exact fp32 copy (fixup)
                nc.sync.dma_start(
                    w1x_s[:], w1x_d.rearrange("(k p) h -> p k h", p=P)
                )
            else:
                w1x_s = w1_s

            yraw = sp.tile([P, NT], F32)

            def emit_transposes(x_t, tf32: bool, tile_idx=None):
                """PE-transpose the 8 K-blocks (fp16 on the tf32 path)."""
                if tf32:
                    xh4 = xh_tiles[tile_idx // 4]
                    src_t = xh4[:, tile_idx % 4, :]
                    iden, tdt = ident16, F16
                else:
                    src_t, iden, tdt = x_t, ident, F32
                xt_s = xtp.tile([P, NK, P], tdt, tag="xt")
                for half in range(2):
                    ps_t = pst.tile([P, 512], tdt, tag="pst")
                    for j in range(4):
                        k = half * 4 + j
                        nc.tensor.transpose(
                            out=ps_t[:, j * P : (j + 1) * P],
                            in_=src_t[:, k * P : (k + 1) * P],
                            identity=iden[:],
                        )
                    nc.scalar.copy(
                        xt_s[:, half * 4 : half * 4 + 4, :].rearrange(
                            "p a b -> p (a b)"
                        ),
                        ps_t[:],
                    )
                return xt_s

            def emit_mms(xt_s, ycol, tf32: bool):
                """K-chunk matmuls + fused relu*W2 rowsum -> ycol."""
                w1t = w1_s if tf32 else w1x_s
                h_ps = psh.tile([P, DH], F32, tag="hps")
                for k in range(NK):
                    nc.tensor.matmul(
                        out=h_ps[:],
                        lhsT=xt_s[:, k, :],
                        rhs=w1t[:, k, :],
                        start=(k == 0),
                        stop=(k == NK - 1) and not b1_nonzero,
                    )
                if b1_nonzero:
                    nc.tensor.matmul(
                        out=h_ps[:],
                        lhsT=ones1x128[:],
                        rhs=b1_s[:],
                        start=False,
                        stop=True,
                    )
                dump = hwp.tile([P, DH], F32, tag="dump")
                nc.vector.scalar_tensor_tensor(
                    out=dump[:],
                    in0=h_ps[:],
                    scalar=0.0,
                    in1=w2_s[:],
                    op0=mybir.AluOpType.max,
                    op1=mybir.AluOpType.mult,
                    accum_out=ycol,
                )

            def compute_y_tile(x_t, ycol, tf32: bool):
                emit_mms(emit_transposes(x_t, tf32), ycol, tf32)

            # ---- phase C: software-pipelined ----
            # Emit transposes of tile i before the matmuls of tile i-1 so the
            # PE never stalls waiting for the ACT PSUM->SBUF copy of the
            # current tile's transposed blocks. The fp32 x tiles are only
            # needed by phase S, so their loads trickle in behind the weights.
            x_tiles = []
            for i in range(NT):
                x_t = xp.tile([P, D], F32, tag="xrow")
                x_tiles.append(x_t)
                nc.sync.dma_start(x_t[:], x_d[i * P : (i + 1) * P, :])
            pending = None
            for i in range(NT):
                xt_s = emit_transposes(x_tiles[i], tf32=USE_TF32, tile_idx=i)
                if pending is not None:
                    emit_mms(pending[0], pending[1], tf32=USE_TF32)
                pending = (xt_s, yraw[:, i : i + 1])
            emit_mms(pending[0], pending[1], tf32=USE_TF32)

            nc.sync.dma_start(yraw_d[:, :], yraw[:])
            stats_s = sp.tile([1, 4], F32)
            nc.vector.memset(stats_s[:], 0.0)

            # ---- tf32 boundary fixup: recompute |y-C|<DELTA tokens in fp32 ----
            if USE_TF32:
                band = sp.tile([P, NT], F32)
                bhi = sp.tile([P, NT], F32)
                nc.vector.scalar_tensor_tensor(
                    out=bhi[:], in0=yraw[:], scalar=RAW_THRESHOLD + DELTA,
                    in1=validf[:],
                    op0=mybir.AluOpType.is_lt, op1=mybir.AluOpType.mult,
                )
                nc.vector.scalar_tensor_tensor(
                    out=band[:], in0=yraw[:], scalar=RAW_THRESHOLD - DELTA,
                    in1=bhi[:],
                    op0=mybir.AluOpType.is_gt, op1=mybir.AluOpType.mult,
                )
                # per-partition inclusive prefix count of band tokens
                psc = sp.tile([P, NT], F32)
                nc.vector.tensor_tensor_scan(
                    out=psc[:], data0=band[:], data1=onespn[:], initial=0.0,
                    op0=mybir.AluOpType.add, op1=mybir.AluOpType.mult,
                )
                # overflow indicator: max band tokens in any partition
                ps_ov = psh.tile([1, P], F32, tag="hps")
                nc.tensor.transpose(
                    out=ps_ov[:], in_=psc[:, NT - 1 : NT], identity=ident[:]
                )
                nc.vector.tensor_reduce(
                    out=stats_s[:, 2:3], in_=ps_ov[:], axis=mybir.AxisListType.X,
                    op=mybir.AluOpType.max,
                )
                for sl in range(FIX_SLOTS):
                    msl = sp.tile([P, NT], F32, tag="msl")
                    nc.vector.scalar_tensor_tensor(
                        out=msl[:], in0=psc[:], scalar=float(sl + 1), in1=band[:],
                        op0=mybir.AluOpType.is_equal, op1=mybir.AluOpType.mult,
                    )
                    # accum(msl * (tid - BIG)) + BIG == token id, or BIG if empty
                    mt = sp.tile([P, NT], F32, tag="mt")
                    idf = sp.tile([P, 1], F32, tag="idf")
                    nc.vector.scalar_tensor_tensor(
                        out=mt[:], in0=msl[:], scalar=1.0, in1=tidb[:],
                        op0=mybir.AluOpType.mult, op1=mybir.AluOpType.mult,
                        accum_out=idf[:],
                    )
                    nc.vector.tensor_scalar(
                        out=idf[:], in0=idf[:], scalar1=BIG, scalar2=None,
                        op0=mybir.AluOpType.add,
                    )
                    idi = sp.tile([P, 1], I32, tag="idi")
                    nc.vector.tensor_copy(idi[:], idf[:])

                    xg = fxp.tile([P, D], F32, tag="xg")
                    nc.gpsimd.memset(xg[:], 0.0)
                    nc.gpsimd.indirect_dma_start(
                        out=xg[:],
                        out_offset=None,
                        in_=x_d[:, :],
                        in_offset=bass.IndirectOffsetOnAxis(ap=idi[:], axis=0),
                        bounds_check=S - 1,
                        oob_is_err=False,
                    )
                    yx = sp.tile([P, 1], F32, tag="yx")
                    compute_y_tile(xg, yx[:], tf32=False)
                    # merge: yraw = yraw*(1-msl) + yx (broadcast per partition)*msl
                    fixv = sp.tile([P, NT], F32, tag="fixv")
                    nc.vector.tensor_scalar_mul(fixv[:], msl[:], yx[:])
                    nc.vector.tensor_scalar(
                        out=msl[:], in0=msl[:], scalar1=-1.0, scalar2=1.0,
                        op0=mybir.AluOpType.mult, op1=mybir.AluOpType.add,
                    )
                    nc.vector.tensor_tensor(
                        out=yraw[:], in0=yraw[:], in1=msl[:],
                        op=mybir.AluOpType.mult,
                    )
                    nc.vector.tensor_tensor(
                        out=yraw[:], in0=yraw[:], in1=fixv[:],
                        op=mybir.AluOpType.add,
                    )

            # ---- phase F: gate, mask, prefix sums, destinations ----
            ygate = sp.tile([P, NT], F32)
            nc.scalar.activation(
                ygate[:], yraw[:], mybir.ActivationFunctionType.Tanh,
                bias=b2x10[:], scale=10.0,
            )
            nc.vector.tensor_scalar(
                out=ygate[:], in0=ygate[:], scalar1=0.5, scalar2=0.5,
                op0=mybir.AluOpType.mult, op1=mybir.AluOpType.add,
            )

            # kept = (yraw > RAW_THRESHOLD) * valid   (pre-tanh, exact boundary)
            kept = sp.tile([P, NT], F32)
            nc.vector.tensor_scalar(
                out=kept[:], in0=yraw[:], scalar1=RAW_THRESHOLD, scalar2=None,
                op0=mybir.AluOpType.is_gt,
            )
            nc.vector.tensor_tensor(
                out=kept[:], in0=kept[:], in1=validf[:], op=mybir.AluOpType.mult
            )

            # gval = gate value for kept tokens (0 elsewhere)
            gval = sp.tile([P, NT], F32)
            nc.vector.tensor_tensor(
                out=gval[:], in0=ygate[:], in1=kept[:], op=mybir.AluOpType.mult
            )

            # row max of gate over valid tokens (for the host adjust check)
            ym = sp.tile([P, NT], F32)
            nc.vector.tensor_tensor(
                out=ym[:], in0=ygate[:], in1=validf[:], op=mybir.AluOpType.mult
            )
            colmax = sp.tile([P, 1], F32)
            nc.vector.tensor_reduce(
                out=colmax[:], in_=ym[:], axis=mybir.AxisListType.X,
                op=mybir.AluOpType.max,
            )
            ps_rm = psh.tile([1, P], F32, tag="hps")
            nc.tensor.transpose(out=ps_rm[:], in_=colmax[:], identity=ident[:])
            nc.vector.tensor_reduce(
                out=stats_s[:, 0:1], in_=ps_rm[:], axis=mybir.AxisListType.X,
                op=mybir.AluOpType.max,
            )

            # prefix sums: incl[p,c] = sum_{k<=p} kept[k,c]
            ps_a = pst.tile([P, NT], F32, tag="pst")
            nc.tensor.matmul(out=ps_a[:], lhsT=ut128[:], rhs=kept[:],
                             start=True, stop=False)
            # column totals on partitions: T[c] = sum_p kept[p,c]
            ps_t32 = psh.tile([NT, 1], F32, tag="hps")
            nc.tensor.matmul(out=ps_t32[:], lhsT=kept[:], rhs=ones128[:],
                             start=True, stop=True)
            t_s = sp.tile([NT, 1], F32)
            nc.scalar.copy(t_s[:], ps_t32[:])
            # col offsets (exclusive) + grand total
            ps_off = psh.tile([1, NT + 1], F32, tag="hps")
            nc.tensor.matmul(out=ps_off[:], lhsT=t_s[:], rhs=sut33[:],
                             start=True, stop=True)
            off_s = sp.tile([1, NT + 1], F32)
            nc.scalar.copy(off_s[:], ps_off[:])
            nc.scalar.copy(stats_s[:, 1:2], off_s[:, NT : NT + 1])
            nc.sync.dma_start(stats_d[:, :], stats_s[:])
            # broadcast col offsets across partitions, accumulated into ps_a
            nc.tensor.matmul(out=ps_a[:], lhsT=ones1x128[:], rhs=off_s[:, :NT],
                             start=False, stop=True)

            # K_t = global exclusive kept-prefix = ps_a - kept
            kt = sp.tile([P, NT], F32)
            nc.vector.tensor_tensor(
                out=kt[:], in0=ps_a[:], in1=kept[:], op=mybir.AluOpType.subtract
            )
            # alt = (S-1) - t + K_t  (rejected tokens fill the tail backwards)
            alt = sp.tile([P, NT], F32)
            nc.vector.tensor_tensor(
                out=alt[:], in0=kt[:], in1=iotar[:], op=mybir.AluOpType.add
            )
            # dst = kept ? K_t : alt  ==  alt - kept*iotar
            ktmp = sp.tile([P, NT], F32)
            nc.vector.tensor_tensor(
                out=ktmp[:], in0=kept[:], in1=iotar[:], op=mybir.AluOpType.mult
            )
            dstf = sp.tile([P, NT], F32)
            nc.vector.tensor_tensor(
                out=dstf[:], in0=alt[:], in1=ktmp[:], op=mybir.AluOpType.subtract
            )
            dsti = sp.tile([P, NT], I32)
            nc.vector.tensor_copy(dsti[:], dstf[:])

            # ---- phase S: gate x in place and scatter token rows ----
            scatter_insts = []
            for i in range(NT):
                x_t = x_tiles[i]
                nc.vector.tensor_scalar_mul(x_t[:], x_t[:], gval[:, i : i + 1])
                r = nc.gpsimd.indirect_dma_start(
                    out=v_d[:, :],
                    out_offset=bass.IndirectOffsetOnAxis(
                        ap=dsti[:, i : i + 1], axis=0
                    ),
                    in_=x_t[:],
                    in_offset=None,
                )
                scatter_insts.append(r.ins if hasattr(r, "ins") else r)

            # The scatters write disjoint rows of v_out (dst is a permutation),
            # but Tile sees whole-tensor WAW and serializes each scatter on the
            # previous one's DMA completion (~3.3us each). Drop those edges.
            snames = {i.name for i in scatter_insts}
            imap = nc.inst_map
            for ins in scatter_insts:
                deps = list(ins.take_sync_dependencies())
                for n in deps:
                    if n not in snames:
                        add_dep_helper(ins, imap[n], True)

    nc.compile()
    return nc


_NC_CACHE = {}


def _get_module(b1_nonzero: bool):
    key = bool(b1_nonzero)
    if key not in _NC_CACHE:
        _NC_CACHE[key] = build_module(key)
    return _NC_CACHE[key]


def _np_row(x_row, pad_row, W1, b1, W2, b2):
    """Exact numpy port of the reference for one row (host fallback)."""
    length = pad_row.size - int(pad_row.sum())
    h = np.maximum(x_row.astype(np.float32) @ W1 + b1, 0.0)
    y = (h @ W2)[:, 0] + b2[0]
    y = (1.0 + np.tanh(10.0 * y)) * 0.5
    pos = np.arange(x_row.shape[0])
    valid = pos < length
    y = np.where(valid, y, 0.0)
    adjust = max(EPS + THRESHOLD - float(y.max()), 0.0)
    y = y + adjust
    u = x_row * y[:, None]
    u_mask = (y > THRESHOLD) & (~pad_row)
    new_len = int(u_mask.sum())
    order = np.argsort(~u_mask, kind="stable")
    v = u[order]
    v[new_len:] = 0.0
    v_pad = pos >= new_len
    return v.astype(np.float32), v_pad, new_len


def kernel(x, pad, W1, b1, W2, b2, _trace=False, _trace_kwargs=None):
    x = np.asarray(x, dtype=np.float32)
    pad = np.asarray(pad).astype(bool)
    W1 = np.asarray(W1, dtype=np.float32)
    b1 = np.asarray(b1, dtype=np.float32)
    W2 = np.asarray(W2, dtype=np.float32)
    b2 = np.asarray(b2, dtype=np.float32)

    b1_nonzero = bool(np.any(b1 != 0))
    nc = _get_module(b1_nonzero)

    consts = _build_consts()
    x16 = x.astype(np.float16) if USE_TF32 else None
    W1r = W1.astype(np.float16) if USE_TF32 else W1
    w2r = np.ascontiguousarray(W2.reshape(1, DH))
    b2r = np.ascontiguousarray(b2.reshape(1, 1))
    in_maps = []
    for b in range(B):
        m = {
            "x_row": np.ascontiguousarray(x[b]),
            **({"x16_row": x16[b]} if USE_TF32 else {}),
            # valid[p, c] corresponds to token t = c*128 + p
            "valid_col": np.ascontiguousarray(
                (~pad[b]).astype(np.float32).reshape(NT, P).T
            ),
            "w1": W1r,
            "w2r": w2r,
            "b2r": b2r,
        }
        if USE_TF32:
            m["w1x"] = W1
        if b1_nonzero:
            m["b1r"] = np.ascontiguousarray(b1.reshape(1, DH))
        m.update(consts)
        in_maps.append(m)

    res = run_bass_kernel_spmd(
        nc,
        in_maps,
        core_ids=list(range(B)),
        trace=_trace,
        **(_trace_kwargs or {}),
    )
    kernel.last_results = res

    v = np.empty((B, S, D), dtype=np.float32)
    v_pad = np.empty((B, S), dtype=bool)
    pos = np.arange(S)
    for b in range(B):
        out = res.results[b]
        rowmax = float(out["stats"][0, 0])
        new_len = int(round(float(out["stats"][0, 1])))
        overflow = float(out["stats"][0, 2])
        if rowmax < THRESHOLD + EPS + 1e-6 or overflow > FIX_SLOTS:
            # adjust lift or fixup overflow -> exact host fallback for this row
            v[b], v_pad[b], _ = _np_row(x[b], pad[b], W1, b1, W2, b2)
        else:
            v[b] = out["v_out"]
            v_pad[b] = pos >= new_len
    return v, v_pad


# revision 14
# speedup vs baseline: 1.0214x; 1.0214x over previous
"""SoftGate Trainium2 kernel.

nn_SoftGate: gate-MLP (Linear->ReLU->Linear->squashed tanh) over [B=8, S=4096,
D=1024] fp32, threshold mask, per-row stable compaction of kept tokens.

Strategy: data-parallel over batch across 8 NeuronCores (one row per core).
On each core:
  - stream x row token-major into SBUF (resident, 16 MB),
  - PE-transpose 128x128 blocks to get D-on-partitions for the matmul,
  - h = relu(x @ W1 (+ b1)) via 8 accumulating matmuls per 128-token tile in
    float32r (tf32, 1 cycle/row; native fp32 is 4 cycles/row on TRN2),
  - fused DVE scalar_tensor_tensor: (h max 0) * W2 with accum_out -> y per token,
  - tf32 boundary fixup: tokens with |y - C| < DELTA (C = threshold in pre-tanh
    space) get y recomputed exactly in native fp32 (indirect-DMA gather of those
    token rows, <= 2 per partition), so the keep/drop decision is fp32-exact,
  - gate y' = (1 + tanh(10 y + 10 b2)) / 2 on ACT; kept mask from pre-tanh y,
  - prefix sums via triangular matmuls on PE -> stable permutation dst (kept
    compact to front, rejected fill the tail backwards with zero values),
  - u = x * gate (DVE per-token scalar), indirect-DMA scatter of 4 KB token
    rows to v[dst].
The 'adjust' lift (only active when no token passes the threshold) is
speculated to be zero; the kernel reports each row's max gate and the fixup
overflow count so the host can fall back to an exact numpy recompute for rows
where speculation fails (never fires for this input distribution).
"""

import os
import sys

import numpy as np

for _p in ("/opt/trn_rl_repo",):
    if _p not in sys.path and os.path.isdir(_p):
        sys.path.insert(0, _p)

import concourse.bass as bass
import concourse.tile as tile
from bass_rust import add_dep_helper
from concourse import bacc, mybir
from concourse.bass_utils import run_bass_kernel_spmd

THRESHOLD = 0.1
EPS = 1e-05
B, S, D = 8, 4096, 1024
DH = 512
P = 128
NT = S // P          # 32 token tiles per row
NK = D // P          # 8 contraction chunks
F32 = mybir.dt.float32
F32R = mybir.dt.float32r
F16 = mybir.dt.float16
I32 = mybir.dt.int32

USE_TF32 = True
FIX_SLOTS = 1        # boundary tokens handled per partition
DELTA = 5e-3         # half-width of the recompute band around the threshold
BIG = 1.0e6          # out-of-bounds marker for empty fixup slots

# kept <=> (1 + tanh(10*y)) / 2 > THRESHOLD <=> y > atanh(2*THRESHOLD - 1) / 10
RAW_THRESHOLD = float(np.arctanh(2.0 * THRESHOLD - 1.0) / 10.0)


def _round_tf32(a):
    """Round-to-nearest-even to 10-bit mantissa (tf32), keeping fp32 storage."""
    v = a.astype(np.float32).view(np.uint32).astype(np.uint64)
    v = (v + 0x1000 + ((v >> 13) & 1)) & ~np.uint64(0x1FFF)
    return (v & 0xFFFFFFFF).astype(np.uint32).view(np.float32)


def _build_consts():
    """Small host-precomputed constant tensors (replicated to every core)."""
    ut128 = np.triu(np.ones((P, P), dtype=np.float32))          # ut[k, m] = 1 if k <= m
    ident = np.eye(P, dtype=np.float32)
    ones128 = np.ones((P, 1), dtype=np.float32)
    ones1x128 = np.ones((1, P), dtype=np.float32)
    # sut33[k, n] = 1 if k < n (n < 32); column 32 = all ones (grand total)
    sut33 = np.zeros((NT, NT + 1), dtype=np.float32)
    sut33[:, :NT] = np.triu(np.ones((NT, NT), dtype=np.float32), k=1)
    sut33[:, NT] = 1.0
    # token id t = c*128 + p laid out [p, c]; iotar = (S-1) - t
    t_idx = (np.arange(NT)[None, :] * P + np.arange(P)[:, None]).astype(np.float32)
    iotar = (S - 1) - t_idx
    onespn = np.ones((P, NT), dtype=np.float32)
    return {
        "c_ut128": ut128,
        "c_ident": ident,
        "c_ident16": ident.astype(np.float16),
        "c_ones128": ones128,
        "c_ones1x128": ones1x128,
        "c_sut33": sut33,
        "c_iotar": iotar,
        "c_tid": t_idx,
        "c_tidb": t_idx - BIG,
        "c_onespn": onespn,
    }


def build_module(b1_nonzero: bool):
    """Trace the single-core bass module. Returns the traced nc."""
    nc = bacc.Bacc(
        "TRN2",
        target_bir_lowering=False,
        debug=False,
        num_devices=8,
    )

    x_d = nc.dram_tensor("x_row", [S, D], F32, kind="ExternalInput")
    x16_d = None
    if USE_TF32:
        x16_d = nc.dram_tensor("x16_row", [S, D], F16, kind="ExternalInput")
    valid_d = nc.dram_tensor("valid_col", [P, NT], F32, kind="ExternalInput")
    # w1: host-rounded to fp16 (the gate matmul runs in fp16: same 11-bit
    # mantissa as tf32, but 2-byte weights halve the LDWEIGHTS cost)
    w1_d = nc.dram_tensor("w1", [D, DH], F16 if USE_TF32 else F32,
                          kind="ExternalInput")
    w1x_d = None
    if USE_TF32:
        w1x_d = nc.dram_tensor("w1x", [D, DH], F32, kind="ExternalInput")
    w2_d = nc.dram_tensor("w2r", [1, DH], F32, kind="ExternalInput")
    b2_d = nc.dram_tensor("b2r", [1, 1], F32, kind="ExternalInput")
    b1_d = None
    if b1_nonzero:
        b1_d = nc.dram_tensor("b1r", [1, DH], F32, kind="ExternalInput")

    c_ut128 = nc.dram_tensor("c_ut128", [P, P], F32, kind="ExternalInput")
    c_ident = nc.dram_tensor("c_ident", [P, P], F32, kind="ExternalInput")
    c_ident16 = nc.dram_tensor("c_ident16", [P, P], F16, kind="ExternalInput")
    c_ones128 = nc.dram_tensor("c_ones128", [P, 1], F32, kind="ExternalInput")
    c_ones1x128 = nc.dram_tensor("c_ones1x128", [1, P], F32, kind="ExternalInput")
    c_sut33 = nc.dram_tensor("c_sut33", [NT, NT + 1], F32, kind="ExternalInput")
    c_iotar = nc.dram_tensor("c_iotar", [P, NT], F32, kind="ExternalInput")
    c_tid = nc.dram_tensor("c_tid", [P, NT], F32, kind="ExternalInput")
    c_tidb = nc.dram_tensor("c_tidb", [P, NT], F32, kind="ExternalInput")
    c_onespn = nc.dram_tensor("c_onespn", [P, NT], F32, kind="ExternalInput")

    v_d = nc.dram_tensor("v_out", [S, D], F32, kind="ExternalOutput")
    stats_d = nc.dram_tensor("stats", [1, 4], F32, kind="ExternalOutput")
    yraw_d = nc.dram_tensor("yraw_out", [P, NT], F32, kind="ExternalOutput")

    with tile.TileContext(nc) as tc:
        with (
            tc.tile_pool(name="xres", bufs=NT) as xp,            # resident x row
            tc.tile_pool(name="consts", bufs=1) as cp,
            tc.tile_pool(name="xt", bufs=3) as xtp,              # transposed x tiles
            tc.tile_pool(name="xh", bufs=2) as xhp,              # fp16 x (4-tile batches)
            tc.tile_pool(name="hw", bufs=2) as hwp,              # fused-op dump
            tc.tile_pool(name="small", bufs=1) as sp,
            tc.tile_pool(name="fix", bufs=FIX_SLOTS) as fxp,     # gathered band rows
            tc.tile_pool(name="pst", bufs=4, space="PSUM") as pst,    # transpose banks
            tc.tile_pool(name="psh", bufs=2, space="PSUM") as psh,    # h accumulate
        ):
            # ---- constants / weights ----
            # Small constants first: the transposes + PE warm-up need ident/
            # ut128 immediately; the 2 MB W1 loads would otherwise block this
            # HWDGE FIFO for ~15us.
            ut128 = cp.tile([P, P], F32)
            nc.sync.dma_start(ut128[:], c_ut128[:, :])
            ident = cp.tile([P, P], F32)
            nc.sync.dma_start(ident[:], c_ident[:, :])
            ident16 = cp.tile([P, P], F16)
            nc.sync.dma_start(ident16[:], c_ident16[:, :])

            # PE warm-up: ~5us of dense matmul trips the HAM clock gate to 8/8
            warm_ps = pst.tile([P, 512], F32, tag="pst")
            for w in range(12):
                nc.tensor.matmul(
                    out=warm_ps[:, :P], lhsT=ident[:], rhs=ut128[:],
                    start=(w == 0), stop=(w == 11),
                )

            # W1 right behind the warm-up consts: the first h-matmul needs it
            w1_s = cp.tile([P, NK, DH], F16 if USE_TF32 else F32)
            w1_r = w1_d.rearrange("(k p) h -> p k h", p=P)
            nc.sync.dma_start(w1_s[:, : NK // 2, :], w1_r[:, : NK // 2, :])
            nc.sync.dma_start(w1_s[:, NK // 2 :, :], w1_r[:, NK // 2 :, :])
            xh_tiles = []
            if USE_TF32:
                # fp16 x in 1 MB batches of 4 token-tiles, straight after W1
                for g in range(NT // 4):
                    xh = xhp.tile([P, 4, D], F16, tag="xh")
                    xh_tiles.append(xh)
                    nc.sync.dma_start(
                        xh[:],
                        x16_d[g * 4 * P : (g + 1) * 4 * P, :].rearrange(
                            "(c p) d -> p c d", p=P
                        ),
                    )

            ones128 = cp.tile([P, 1], F32)
            nc.sync.dma_start(ones128[:], c_ones128[:, :])
            ones1x128 = cp.tile([1, P], F32)
            nc.sync.dma_start(ones1x128[:], c_ones1x128[:, :])
            sut33 = cp.tile([NT, NT + 1], F32)
            nc.sync.dma_start(sut33[:], c_sut33[:, :])
            iotar = cp.tile([P, NT], F32)
            nc.sync.dma_start(iotar[:], c_iotar[:, :])
            tid = cp.tile([P, NT], F32)
            nc.sync.dma_start(tid[:], c_tid[:, :])
            tidb = cp.tile([P, NT], F32)
            nc.sync.dma_start(tidb[:], c_tidb[:, :])
            onespn = cp.tile([P, NT], F32)
            nc.sync.dma_start(onespn[:], c_onespn[:, :])
            validf = cp.tile([P, NT], F32)
            nc.sync.dma_start(validf[:], valid_d[:, :])
            w2_s = cp.tile([P, DH], F32)              # W2 replicated on partitions
            nc.sync.dma_start(w2_s[:], w2_d[:, :].to_broadcast([P, DH]))
            b2_s = cp.tile([P, 1], F32)
            nc.sync.dma_start(b2_s[:], b2_d[:, :].to_broadcast([P, 1]))
            b2x10 = cp.tile([P, 1], F32)
            nc.scalar.mul(b2x10[:], b2_s[:], 10.0)
            if b1_nonzero:
                b1_s = cp.tile([1, DH], F32)
                nc.sync.dma_start(b1_s[:], b1_d[:, :])

            if USE_TF32:
                w1x_s = cp.tile([P, NK, DH], F32)     # exact fp32 copy (fixup)
                nc.sync.dma_start(
                    w1x_s[:], w1x_d.rearrange("(k p) h -> p k h", p=P)
                )
            else:
                w1x_s = w1_s

            yraw = sp.tile([P, NT], F32)
            xg_tiles = []
            if USE_TF32:
                for _sl in range(FIX_SLOTS):
                    _xg = fxp.tile([P, D], F32, tag="xg")
                    xg_tiles.append(_xg)
                    nc.gpsimd.memset(_xg[:], 0.0)

            def emit_transposes(x_t, tf32: bool, tile_idx=None):
                """PE-transpose the 8 K-blocks (fp16 on the tf32 path)."""
                if tf32:
                    xh4 = xh_tiles[tile_idx // 4]
                    src_t = xh4[:, tile_idx % 4, :]
                    iden, tdt = ident16, F16
                else:
                    src_t, iden, tdt = x_t, ident, F32
                xt_s = xtp.tile([P, NK, P], tdt, tag="xt")
                for half in range(2):
                    ps_t = pst.tile([P, 512], tdt, tag="pst")
                    for j in range(4):
                        k = half * 4 + j
                        nc.tensor.transpose(
                            out=ps_t[:, j * P : (j + 1) * P],
                            in_=src_t[:, k * P : (k + 1) * P],
                            identity=iden[:],
                        )
                    nc.scalar.copy(
                        xt_s[:, half * 4 : half * 4 + 4, :].rearrange(
                            "p a b -> p (a b)"
                        ),
                        ps_t[:],
                    )
                return xt_s

            def emit_mms(xt_s, ycol, tf32: bool):
                """K-chunk matmuls + fused relu*W2 rowsum -> ycol."""
                w1t = w1_s if tf32 else w1x_s
                h_ps = psh.tile([P, DH], F32, tag="hps")
                for k in range(NK):
                    nc.tensor.matmul(
                        out=h_ps[:],
                        lhsT=xt_s[:, k, :],
                        rhs=w1t[:, k, :],
                        start=(k == 0),
                        stop=(k == NK - 1) and not b1_nonzero,
                    )
                if b1_nonzero:
                    nc.tensor.matmul(
                        out=h_ps[:],
                        lhsT=ones1x128[:],
                        rhs=b1_s[:],
                        start=False,
                        stop=True,
                    )
                dump = hwp.tile([P, DH], F32, tag="dump")
                nc.vector.scalar_tensor_tensor(
                    out=dump[:],
                    in0=h_ps[:],
                    scalar=0.0,
                    in1=w2_s[:],
                    op0=mybir.AluOpType.max,
                    op1=mybir.AluOpType.mult,
                    accum_out=ycol,
                )

            def compute_y_tile(x_t, ycol, tf32: bool):
                emit_mms(emit_transposes(x_t, tf32), ycol, tf32)

            # ---- phase C: software-pipelined ----
            # Emit transposes of tile i before the matmuls of tile i-1 so the
            # PE never stalls waiting for the ACT PSUM->SBUF copy of the
            # current tile's transposed blocks. The fp32 x tiles are only
            # needed by phase S, so their loads trickle in behind the weights.
            x_tiles = []
            for i in range(NT):
                x_t = xp.tile([P, D], F32, tag="xrow")
                x_tiles.append(x_t)
                nc.sync.dma_start(x_t[:], x_d[i * P : (i + 1) * P, :])
            pending = None
            for i in range(NT):
                xt_s = emit_transposes(x_tiles[i], tf32=USE_TF32, tile_idx=i)
                if pending is not None:
                    emit_mms(pending[0], pending[1], tf32=USE_TF32)
                pending = (xt_s, yraw[:, i : i + 1])
            emit_mms(pending[0], pending[1], tf32=USE_TF32)

            nc.sync.dma_start(yraw_d[:, :], yraw[:])
            stats_s = sp.tile([1, 4], F32)
            nc.vector.memset(stats_s[:], 0.0)

            # ---- tf32 boundary fixup: recompute |y-C|<DELTA tokens in fp32 ----
            if USE_TF32:
                band = sp.tile([P, NT], F32)
                bhi = sp.tile([P, NT], F32)
                nc.vector.scalar_tensor_tensor(
                    out=bhi[:], in0=yraw[:], scalar=RAW_THRESHOLD + DELTA,
                    in1=validf[:],
                    op0=mybir.AluOpType.is_lt, op1=mybir.AluOpType.mult,
                )
                nc.vector.scalar_tensor_tensor(
                    out=band[:], in0=yraw[:], scalar=RAW_THRESHOLD - DELTA,
                    in1=bhi[:],
                    op0=mybir.AluOpType.is_gt, op1=mybir.AluOpType.mult,
                )
                # per-partition inclusive prefix count of band tokens
                psc = sp.tile([P, NT], F32)
                nc.vector.tensor_tensor_scan(
                    out=psc[:], data0=band[:], data1=onespn[:], initial=0.0,
                    op0=mybir.AluOpType.add, op1=mybir.AluOpType.mult,
                )
                # overflow indicator: max band tokens in any partition
                ps_ov = psh.tile([1, P], F32, tag="hps")
                nc.tensor.transpose(
                    out=ps_ov[:], in_=psc[:, NT - 1 : NT], identity=ident[:]
                )
                nc.vector.tensor_reduce(
                    out=stats_s[:, 2:3], in_=ps_ov[:], axis=mybir.AxisListType.X,
                    op=mybir.AluOpType.max,
                )
                for sl in range(FIX_SLOTS):
                    msl = sp.tile([P, NT], F32, tag="msl")
                    nc.vector.scalar_tensor_tensor(
                        out=msl[:], in0=psc[:], scalar=float(sl + 1), in1=band[:],
                        op0=mybir.AluOpType.is_equal, op1=mybir.AluOpType.mult,
                    )
                    # accum(msl * (tid - BIG)) + BIG == token id, or BIG if empty
                    mt = sp.tile([P, NT], F32, tag="mt")
                    idf = sp.tile([P, 1], F32, tag="idf")
                    nc.vector.scalar_tensor_tensor(
                        out=mt[:], in0=msl[:], scalar=1.0, in1=tidb[:],
                        op0=mybir.AluOpType.mult, op1=mybir.AluOpType.mult,
                        accum_out=idf[:],
                    )
                    nc.vector.tensor_scalar(
                        out=idf[:], in0=idf[:], scalar1=BIG, scalar2=None,
                        op0=mybir.AluOpType.add,
                    )
                    idi = sp.tile([P, 1], I32, tag="idi")
                    nc.vector.tensor_copy(idi[:], idf[:])

                    xg = xg_tiles[sl]
                    nc.gpsimd.indirect_dma_start(
                        out=xg[:],
                        out_offset=None,
                        in_=x_d[:, :],
                        in_offset=bass.IndirectOffsetOnAxis(ap=idi[:], axis=0),
                        bounds_check=S - 1,
                        oob_is_err=False,
                    )
                    yx = sp.tile([P, 1], F32, tag="yx")
                    compute_y_tile(xg, yx[:], tf32=False)
                    # merge: yraw = yraw*(1-msl) + yx (broadcast per partition)*msl
                    fixv = sp.tile([P, NT], F32, tag="fixv")
                    nc.vector.tensor_scalar_mul(fixv[:], msl[:], yx[:])
                    nc.vector.tensor_scalar(
                        out=msl[:], in0=msl[:], scalar1=-1.0, scalar2=1.0,
                        op0=mybir.AluOpType.mult, op1=mybir.AluOpType.add,
                    )
                    nc.vector.tensor_tensor(
                        out=yraw[:], in0=yraw[:], in1=msl[:],
                        op=mybir.AluOpType.mult,
                    )
                    nc.vector.tensor_tensor(
                        out=yraw[:], in0=yraw[:], in1=fixv[:],
                        op=mybir.AluOpType.add,
                    )

            # ---- phase F: gate, mask, prefix sums, destinations ----
            ygate = sp.tile([P, NT], F32)
            nc.scalar.activation(
                ygate[:], yraw[:], mybir.ActivationFunctionType.Tanh,
                bias=b2x10[:], scale=10.0,
            )
            nc.vector.tensor_scalar(
                out=ygate[:], in0=ygate[:], scalar1=0.5, scalar2=0.5,
                op0=mybir.AluOpType.mult, op1=mybir.AluOpType.add,
            )

            # kept = (yraw > RAW_THRESHOLD) * valid   (pre-tanh, exact boundary)
            kept = sp.tile([P, NT], F32)
            nc.vector.scalar_tensor_tensor(
                out=kept[:], in0=yraw[:], scalar=RAW_THRESHOLD, in1=validf[:],
                op0=mybir.AluOpType.is_gt, op1=mybir.AluOpType.mult,
            )

            # gval = gate value for kept tokens (0 elsewhere)
            gval = sp.tile([P, NT], F32)
            nc.vector.tensor_tensor(
                out=gval[:], in0=ygate[:], in1=kept[:], op=mybir.AluOpType.mult
            )

            # row max of gate over valid tokens (for the host adjust check)
            ym = sp.tile([P, NT], F32)
            nc.vector.tensor_tensor(
                out=ym[:], in0=ygate[:], in1=validf[:], op=mybir.AluOpType.mult
            )
            colmax = sp.tile([P, 1], F32)
            nc.vector.tensor_reduce(
                out=colmax[:], in_=ym[:], axis=mybir.AxisListType.X,
                op=mybir.AluOpType.max,
            )
            ps_rm = psh.tile([1, P], F32, tag="hps")
            nc.tensor.transpose(out=ps_rm[:], in_=colmax[:], identity=ident[:])
            nc.vector.tensor_reduce(
                out=stats_s[:, 0:1], in_=ps_rm[:], axis=mybir.AxisListType.X,
                op=mybir.AluOpType.max,
            )

            # prefix sums: incl[p,c] = sum_{k<=p} kept[k,c]
            ps_a = pst.tile([P, NT], F32, tag="pst")
            nc.tensor.matmul(out=ps_a[:], lhsT=ut128[:], rhs=kept[:],
                             start=True, stop=False)
            # column totals on partitions: T[c] = sum_p kept[p,c]
            ps_t32 = psh.tile([NT, 1], F32, tag="hps")
            nc.tensor.matmul(out=ps_t32[:], lhsT=kept[:], rhs=ones128[:],
                             start=True, stop=True)
            t_s = sp.tile([NT, 1], F32)
            nc.scalar.copy(t_s[:], ps_t32[:])
            # col offsets (exclusive) + grand total
            ps_off = psh.tile([1, NT + 1], F32, tag="hps")
            nc.tensor.matmul(out=ps_off[:], lhsT=t_s[:], rhs=sut33[:],
                             start=True, stop=True)
            off_s = sp.tile([1, NT + 1], F32)
            nc.scalar.copy(off_s[:], ps_off[:])
            nc.scalar.copy(stats_s[:, 1:2], off_s[:, NT : NT + 1])
            nc.sync.dma_start(stats_d[:, :], stats_s[:])
            # broadcast col offsets across partitions, accumulated into ps_a
            nc.tensor.matmul(out=ps_a[:], lhsT=ones1x128[:], rhs=off_s[:, :NT],
                             start=False, stop=True)

            # K_t = global exclusive kept-prefix = ps_a - kept
            kt = sp.tile([P, NT], F32)
            nc.vector.tensor_tensor(
                out=kt[:], in0=ps_a[:], in1=kept[:], op=mybir.AluOpType.subtract
            )
            # alt = (S-1) - t + K_t  (rejected tokens fill the tail backwards)
            alt = sp.tile([P, NT], F32)
            nc.vector.tensor_tensor(
                out=alt[:], in0=kt[:], in1=iotar[:], op=mybir.AluOpType.add
            )
            # dst = kept ? K_t : alt  ==  alt - kept*iotar
            ktmp = sp.tile([P, NT], F32)
            nc.vector.tensor_tensor(
                out=ktmp[:], in0=kept[:], in1=iotar[:], op=mybir.AluOpType.mult
            )
            dstf = sp.tile([P, NT], F32)
            nc.vector.tensor_tensor(
                out=dstf[:], in0=alt[:], in1=ktmp[:], op=mybir.AluOpType.subtract
            )
            dsti = sp.tile([P, NT], I32)
            nc.vector.tensor_copy(dsti[:], dstf[:])

            # ---- phase S: gate x in place and scatter token rows ----
            scatter_insts = []
            for i in range(NT):
                x_t = x_tiles[i]
                nc.vector.tensor_scalar_mul(x_t[:], x_t[:], gval[:, i : i + 1])
                r = nc.gpsimd.indirect_dma_start(
                    out=v_d[:, :],
                    out_offset=bass.IndirectOffsetOnAxis(
                        ap=dsti[:, i : i + 1], axis=0
                    ),
                    in_=x_t[:],
                    in_offset=None,
                )
                scatter_insts.append(r.ins if hasattr(r, "ins") else r)

            # The scatters write disjoint rows of v_out (dst is a permutation),
            # but Tile sees whole-tensor WAW and serializes each scatter on the
            # previous one's DMA completion (~3.3us each). Drop those edges.
            snames = {i.name for i in scatter_insts}
            imap = nc.inst_map
            for ins in scatter_insts:
                deps = list(ins.take_sync_dependencies())
                for n in deps:
                    if n not in snames:
                        add_dep_helper(ins, imap[n], True)

    nc.compile()
    return nc


_NC_CACHE = {}


def _get_module(b1_nonzero: bool):
    key = bool(b1_nonzero)
    if key not in _NC_CACHE:
        _NC_CACHE[key] = build_module(key)
    return _NC_CACHE[key]


def _np_row(x_row, pad_row, W1, b1, W2, b2):
    """Exact numpy port of the reference for one row (host fallback)."""
    length = pad_row.size - int(pad_row.sum())
    h = np.maximum(x_row.astype(np.float32) @ W1 + b1, 0.0)
    y = (h @ W2)[:, 0] + b2[0]
    y = (1.0 + np.tanh(10.0 * y)) * 0.5
    pos = np.arange(x_row.shape[0])
    valid = pos < length
    y = np.where(valid, y, 0.0)
    adjust = max(EPS + THRESHOLD - float(y.max()), 0.0)
    y = y + adjust
    u = x_row * y[:, None]
    u_mask = (y > THRESHOLD) & (~pad_row)
    new_len = int(u_mask.sum())
    order = np.argsort(~u_mask, kind="stable")
    v = u[order]
    v[new_len:] = 0.0
    v_pad = pos >= new_len
    return v.astype(np.float32), v_pad, new_len


def kernel(x, pad, W1, b1, W2, b2, _trace=False, _trace_kwargs=None):
    x = np.asarray(x, dtype=np.float32)
    pad = np.asarray(pad).astype(bool)
    W1 = np.asarray(W1, dtype=np.float32)
    b1 = np.asarray(b1, dtype=np.float32)
    W2 = np.asarray(W2, dtype=np.float32)
    b2 = np.asarray(b2, dtype=np.float32)

    b1_nonzero = bool(np.any(b1 != 0))
    nc = _get_module(b1_nonzero)

    consts = _build_consts()
    x16 = x.astype(np.float16) if USE_TF32 else None
    W1r = W1.astype(np.float16) if USE_TF32 else W1
    w2r = np.ascontiguousarray(W2.reshape(1, DH))
    b2r = np.ascontiguousarray(b2.reshape(1, 1))
    in_maps = []
    for b in range(B):
        m = {
            "x_row": np.ascontiguousarray(x[b]),
            **({"x16_row": x16[b]} if USE_TF32 else {}),
            # valid[p, c] corresponds to token t = c*128 + p
            "valid_col": np.ascontiguousarray(
                (~pad[b]).astype(np.float32).reshape(NT, P).T
            ),
            "w1": W1r,
            "w2r": w2r,
            "b2r": b2r,
        }
        if USE_TF32:
            m["w1x"] = W1
        if b1_nonzero:
            m["b1r"] = np.ascontiguousarray(b1.reshape(1, DH))
        m.update(consts)
        in_maps.append(m)

    res = run_bass_kernel_spmd(
        nc,
        in_maps,
        core_ids=list(range(B)),
        trace=_trace,
        **(_trace_kwargs or {}),
    )
    kernel.last_results = res

    v = np.empty((B, S, D), dtype=np.float32)
    v_pad = np.empty((B, S), dtype=bool)
    pos = np.arange(S)
    for b in range(B):
        out = res.results[b]
        rowmax = float(out["stats"][0, 0])
        new_len = int(round(float(out["stats"][0, 1])))
        overflow = float(out["stats"][0, 2])
        if rowmax < THRESHOLD + EPS + 1e-6 or overflow > FIX_SLOTS:
            # adjust lift or fixup overflow -> exact host fallback for this row
            v[b], v_pad[b], _ = _np_row(x[b], pad[b], W1, b1, W2, b2)
        else:
            v[b] = out["v_out"]
            v_pad[b] = pos >= new_len
    return v, v_pad
